# revision 49
# baseline (speedup 1.0000x reference)
"""GCNEncoder (GCNConv + TransformerEncoderLayer) on 8 Trainium2 NeuronCores.

Sharding: nodes are split 512/core (8 cores). Per core:
  - GCN: dense normalized-adjacency block A^T [4096 src, 512 dst] built on
    device via GPSIMD local_scatter from host-permuted (index-only) edge
    layouts; aggregation is a dense fp16 matmul against replicated scaled
    features, pipelined per src k-tile against scatter completion.
  - Attention: both heads, q = the core's 512 nodes vs all 4096 keys.
    K^T is AllGathered first and S/exp stream against it while the V
    AllGather is still in flight; PV matmuls join the stream once V lands
    (software-pipelined schedule, in-order PE friendly). Softmax skips
    max-subtraction; denominators via one ones-matmul per head over a
    Vector-accumulated exp-sum.
  - FFN + both LayerNorms local, LN pipelined per 128-node chunk.
All rsqrt computed as exp(-0.5*ln(x)) so one activation table serves the
whole kernel. All matmul operands fp16, accumulation fp32 in PSUM.
"""

import math

import numpy as np

import concourse.bacc as bacc
import concourse.mybir as mybir
import concourse.tile as tile
from concourse import library_config
from concourse.tile_rust import add_dep_helper

N_CORES = 8
N = 4096
E = 131072
DIN = 512
D = 256
H = 2
DH = 128
DFF = 2048
EPS = 1e-5
P = 128

NPC = N // N_CORES          # nodes per core = 512
MPC = NPC // P              # m-chunks per core = 4
KT = N // P                 # src k-tiles = 32
KPAD = 32                   # max out-edges per (core, src-node)
KBD = 80                    # max in-edges per dst node
NDUP = 256                  # max duplicate-edge occurrences per core
LPRE = 11                   # attention S/exp prefix depth (kt2 units)
DT16 = mybir.dt.float16
DT8 = mybir.dt.float8e4
DT32 = mybir.dt.float32
DTI16 = mybir.dt.int16
F = mybir.ActivationFunctionType
A = mybir.AluOpType
INV_SQRT_DH = 1.0 / math.sqrt(DH)


def build_kernel():
    nc = bacc.Bacc("TRN2", target_bir_lowering=False, debug=False,
                   num_devices=N_CORES)

    def din(name, shape, dt=DT32):
        return nc.dram_tensor(name, shape, dt, kind="ExternalInput")

    xT_d = din("xT", [P, MPC * DIN], DT16)
    xTf_d = din("xTf", [P, KT * MPC * P], DT16)     # full x.T, j-major wrap
    wbdf_d = din("wbdf", [P, (N // P) * KBD], DT16)  # full per-dst weights
    wg_d = din("wg", [P, (DIN // P) * D], DT16)
    warr_d = din("warr", [P, KT * KPAD], DT16)
    idx_d = din("idx", [P, KT * KPAD], DTI16)
    wbd_d = din("wbd", [P, MPC * KBD], DT16)
    ident_d = din("ident", [P, P])
    winT_d = din("winT", [P, 2 * 3 * D], DT16)
    ipb_d = din("ipb", [P, 6])
    woT_d = din("woT", [P, 2 * D], DT16)
    w1T_d = din("w1T", [P, 2 * DFF], DT16)
    b1_d = din("b1", [P, DFF // P])
    w2T_d = din("w2T", [P, (DFF // P) * D], DT16)
    bias_d = din("bias", [P, 7 * D])                # host-replicated rows

    out_d = nc.dram_tensor("out", [NPC, D], DT32, kind="ExternalOutput")

    with tile.TileContext(nc) as tc:
        with (
            tc.tile_pool(name="keep", bufs=1) as keep,
            tc.tile_pool(name="dram", bufs=1, space="DRAM") as dram,
        ):
            ones16_col = keep.tile([P, 1], DT16)
            nc.vector.memset(ones16_col[:], 1.0)

            lib = nc.gpsimd.load_library(library_config.local_scatter)

            gk = ctx_gcn = tc.tile_pool(name="gcn_keep", bufs=1)
            gk = ctx_gcn.__enter__()

            # warmup collective: absorbs the CC engine's first-program setup
            # cost long before the gathers that matter
            cc_warm = keep.tile([P, 2], DT16)
            nc.vector.memset(cc_warm[:], 0.0)
            warm_b = dram.tile([P, 2], DT16)
            warm_g = dram.tile([N_CORES * P, 2], DT16, addr_space="Shared")
            nc.scalar.dma_start(warm_b[:], cc_warm[:])
            nc.gpsimd.collective_compute(
                "AllGather", A.bypass,
                replica_groups=[list(range(N_CORES))],
                ins=[warm_b.opt()], outs=[warm_g.opt()])

            # ---- input DMAs in consumption order: scatter feeds first,
            # then xw operands, degrees, then the bulk x stream in chunks
            # so matmuls start while later chunks are still in flight ----
            warr = gk.tile([P, KT * KPAD], DT16)
            idx_t = gk.tile([P, KT * KPAD], DTI16)
            nc.sync.dma_start(warr[:], warr_d[:])
            nc.sync.dma_start(idx_t[:], idx_d[:])
            xT16 = gk.tile([P, MPC * DIN], DT16)
            wg16 = gk.tile([P, (DIN // P) * D], DT16)
            nc.sync.dma_start(wg16[:], wg_d[:])
            nc.sync.dma_start(xT16[:], xT_d[:])
            wbd = gk.tile([P, MPC * KBD], DT16)
            nc.sync.dma_start(wbd[:], wbd_d[:])
            wbdf = gk.tile([P, (N // P) * KBD], DT16)
            nc.sync.dma_start(wbdf[:], wbdf_d[:])
            # xTf chunks on the scalar queue: descriptor generation runs in
            # parallel with the sync queue's smaller input DMAs
            xTf16 = gk.tile([P, KT * MPC * P], DT16)
            XCH = 2048
            for g in range(KT * MPC * P // XCH):
                nc.scalar.dma_start(xTf16[:, XCH * g:XCH * (g + 1)],
                                    xTf_d[:, XCH * g:XCH * (g + 1)])

            # fused scatters: two src k-tiles per call (host pre-offsets the
            # odd tile's indices by 512), halving per-call overhead
            a_tiles = [gk.tile([P, 2 * NPC], DT16, tag=f"A{t}", name=f"A{t}")
                       for t in range(KT // 2)]
            scatter_ins = []
            for t in range(KT // 2):
                ls = nc.gpsimd.local_scatter(
                    a_tiles[t][:],
                    warr[:, 2 * KPAD * t:2 * KPAD * (t + 1)],
                    idx_t[:, 2 * KPAD * t:2 * KPAD * (t + 1)],
                    channels=P, num_elems=2 * NPC, num_idxs=2 * KPAD,
                )
                add_dep_helper(ls.ins, lib.ins, reason="scatter after lib")
                scatter_ins.append(ls)

            # ---- degrees -> dinv (local + full) ----
            dinv = gk.tile([P, MPC], DT32)
            dinv2 = gk.tile([P, MPC], DT32)
            deg = gk.tile([P, MPC], DT32)
            nc.vector.tensor_reduce(
                deg[:], wbd[:].rearrange("p (m k) -> p m k", k=KBD),
                axis=mybir.AxisListType.X, op=A.add)
            sqd = gk.tile([P, MPC], DT32)
            nc.scalar.activation(sqd[:], deg[:], F.Sqrt, bias=1.0, scale=1.0)
            nc.vector.reciprocal(dinv[:], sqd[:])
            nc.vector.tensor_mul(dinv2[:], dinv[:], dinv[:])

            dinvf = gk.tile([P, N // P], DT32)
            degf = gk.tile([P, N // P], DT32)
            nc.vector.tensor_reduce(
                degf[:], wbdf[:].rearrange("p (j k) -> p j k", k=KBD),
                axis=mybir.AxisListType.X, op=A.add)
            sqdf = gk.tile([P, N // P], DT32)
            nc.scalar.activation(sqdf[:], degf[:], F.Sqrt, bias=1.0, scale=1.0)
            nc.vector.reciprocal(dinvf[:], sqdf[:])

            # ---- xw = x @ W_gcn: local self-term first, then full ----
            xws16f = gk.tile([P, (N // P) * D], DT16)
            self32 = gk.tile([P, MPC * D], DT32)
            # aggregation PSUM banks live through the interleaved loop and
            # the epilogue below
            ctx_aps = tc.tile_pool(name="agg_ps", bufs=1, space="PSUM")
            aps = ctx_aps.__enter__()
            agg_ps = [aps.tile([P, D], DT32, space="PSUM",
                               tag=f"agg{m}", name=f"agg{m}")
                      for m in range(MPC)]

            def emit_agg(kt):
                for m in range(MPC):
                    agg_mm = nc.tensor.matmul(
                        agg_ps[m][:],
                        lhsT=a_tiles[kt // 2][:, NPC * (kt % 2) + P * m:
                                              NPC * (kt % 2) + P * (m + 1)],
                        rhs=xws16f[:, D * kt:D * (kt + 1)],
                        start=(kt == 0), stop=(kt == KT - 1))
                    if m == 0:
                        add_dep_helper(agg_mm.ins, scatter_ins[kt // 2].ins,
                                       reason="agg kt after scatter kt")

            # xw[j] streams against its xTf chunk; agg[j-2] fills the PE
            # bubbles (its scatter and xws land at a matching pace)
            LAG = 2
            with tc.tile_pool(name="xw_ps", bufs=4, space="PSUM") as xps:
                for m in range(MPC):
                    pxw = xps.tile([P, D], DT32, space="PSUM", tag="xw")
                    for k in range(DIN // P):
                        nc.tensor.matmul(
                            pxw[:],
                            lhsT=xT16[:, DIN * k + P * m:DIN * k + P * m + P],
                            rhs=wg16[:, D * k:D * (k + 1)],
                            start=(k == 0), stop=(k == DIN // P - 1))
                    nc.vector.tensor_scalar(self32[:, D * m:D * (m + 1)], pxw[:],
                                            dinv2[:, m:m + 1], None, op0=A.mult)
                for j in range(N // P):
                    # agg first: if xw[j] stalls on its xTf chunk, the
                    # already-satisfied agg matmuls aren't stuck behind it
                    if j >= LAG:
                        emit_agg(j - LAG)
                    pxw = xps.tile([P, D], DT32, space="PSUM", tag="xw")
                    for k in range(DIN // P):
                        nc.tensor.matmul(
                            pxw[:],
                            lhsT=xTf16[:, MPC * P * j + P * k:
                                       MPC * P * j + P * (k + 1)],
                            rhs=wg16[:, D * k:D * (k + 1)],
                            start=(k == 0), stop=(k == DIN // P - 1))
                    nc.vector.tensor_scalar(xws16f[:, D * j:D * (j + 1)],
                                            pxw[:], dinvf[:, j:j + 1], None,
                                            op0=A.mult)
                for kt in range(N // P - LAG, N // P):
                    emit_agg(kt)

            # constants for later phases (DMA after critical ones)
            ident = keep.tile([P, P], DT32)
            ipb = keep.tile([P, 6], DT32)
            b1t = keep.tile([P, DFF // P], DT32)
            nc.sync.dma_start(ident[:], ident_d[:])
            nc.sync.dma_start(ipb[:], ipb_d[:])
            nc.sync.dma_start(b1t[:], b1_d[:])
            winT16 = keep.tile([P, 2 * 3 * D], DT16)
            woT16 = keep.tile([P, 2 * D], DT16)
            nc.sync.dma_start(winT16[:], winT_d[:])
            nc.sync.dma_start(woT16[:], woT_d[:])
            bias_bc = keep.tile([P, 7 * D], DT32)
            nc.sync.dma_start(bias_bc[:], bias_d[:])
            bgcn_bc = bias_bc[:, 0:D]
            b2_bc = bias_bc[:, D:2 * D]
            ln1g_bc = bias_bc[:, 2 * D:3 * D]
            ln1b_bc = bias_bc[:, 3 * D:4 * D]
            ln2g_bc = bias_bc[:, 4 * D:5 * D]
            ln2b_bc = bias_bc[:, 5 * D:6 * D]
            bo_bc = bias_bc[:, 6 * D:7 * D]

            def bc4(ap_2d):
                """[128, D] bias slice -> broadcast [128, MPC, D]."""
                return ap_2d[:, None, :].to_broadcast([P, MPC, D])

            # ---- GCN epilogue ----
            h_t = keep.tile([P, MPC * D], DT32)
            hT16 = keep.tile([P, 2 * NPC], DT16)
            with tc.tile_pool(name="agg_sb", bufs=2) as asb:
                # h = relu(dinv*agg + self + b_gcn), per-m so transposes start
                # as soon as the first chunk is through the epilogue
                with tc.tile_pool(name="tr_ps", bufs=2, space="PSUM") as tps:
                    for m in range(MPC):
                        x_m = asb.tile([P, D], DT32, tag="xm")
                        nc.vector.scalar_tensor_tensor(
                            x_m[:], agg_ps[m][:],
                            dinv[:, m:m + 1], self32[:, D * m:D * (m + 1)],
                            op0=A.mult, op1=A.add)
                        nc.vector.tensor_tensor(x_m[:], x_m[:], bgcn_bc,
                                                op=A.add)
                        nc.scalar.activation(h_t[:, D * m:D * (m + 1)],
                                             x_m[:], F.Relu)
                        for f in range(2):
                            ptr = tps.tile([P, P], DT32, space="PSUM",
                                           tag="tr")
                            nc.tensor.transpose(
                                ptr[:],
                                h_t[:, D * m + P * f:D * m + P * (f + 1)],
                                ident[:])
                            nc.vector.tensor_copy(
                                hT16[:, NPC * f + P * m:NPC * f + P * (m + 1)],
                                ptr[:])
            ctx_aps.__exit__(None, None, None)
            # prefetch the exp activation table during the collective window
            dummy_e = keep.tile([P, 1], DT32)
            nc.scalar.activation(dummy_e[:], dinv[:, 0:1], F.Exp)

            ctx_gcn.__exit__(None, None, None)
            ak = ctx_attn = tc.tile_pool(name="attn_keep", bufs=1)
            ak = ctx_attn.__enter__()

            # ---- K^T + V packed fp8, ONE AllGather (the mesh collective is
            # latency-dominated, so one 2MB gather beats two serial halves);
            # Q last. Scores/PV feed a softmax so fp8 washes out. ----
            kv_sb = ak.tile([P, 4 * NPC], DT8)
            qT16 = ak.tile([P, H * NPC], DT8)
            with tc.tile_pool(name="kv_ps", bufs=3, space="PSUM") as kvps:
                for h in range(H):
                    pk = kvps.tile([P, NPC], DT32, space="PSUM", tag="kv")
                    for k in range(2):
                        nc.tensor.matmul(
                            pk[:],
                            lhsT=winT16[:, 768 * k + D + P * h:
                                        768 * k + D + P * (h + 1)],
                            rhs=hT16[:, NPC * k:NPC * (k + 1)],
                            start=(k == 0), stop=(k == 1))
                    nc.vector.tensor_scalar(
                        kv_sb[:, NPC * h:NPC * (h + 1)], pk[:],
                        ipb[:, 2 + h:3 + h], None, op0=A.add)

                for h in range(H):
                    for m in range(MPC):
                        pv = kvps.tile([P, P], DT32, space="PSUM", tag="kvv")
                        for k in range(2):
                            nc.tensor.matmul(
                                pv[:],
                                lhsT=hT16[:, NPC * k + P * m:NPC * k + P * (m + 1)],
                                rhs=winT16[:, 768 * k + 2 * D + P * h:
                                            768 * k + 2 * D + P * (h + 1)],
                                start=(k == 0), stop=(k == 1))
                        nc.vector.tensor_copy(
                            kv_sb[:, NPC * (2 + h) + P * m:
                                  NPC * (2 + h) + P * (m + 1)],
                            pv[:])

                kv_bounce = dram.tile([4 * P, NPC], DT8)
                kv_gath = dram.tile([N_CORES * 4 * P, NPC], DT8,
                                    addr_space="Shared")
                nc.scalar.dma_start(
                    kv_bounce[:].rearrange("(x p) n -> p x n", p=P),
                    kv_sb[:].rearrange("p (x n) -> p x n", x=4))
                nc.gpsimd.collective_compute(
                    "AllGather", A.bypass,
                    replica_groups=[list(range(N_CORES))],
                    ins=[kv_bounce.opt()], outs=[kv_gath.opt()])

                for h in range(H):
                    pq = kvps.tile([P, NPC], DT32, space="PSUM", tag="kv")
                    for k in range(2):
                        nc.tensor.matmul(
                            pq[:],
                            lhsT=winT16[:, 768 * k + P * h:768 * k + P * (h + 1)],
                            rhs=hT16[:, NPC * k:NPC * (k + 1)],
                            start=(k == 0), stop=(k == 1))
                    nc.vector.tensor_scalar(
                        qT16[:, NPC * h:NPC * (h + 1)], pq[:],
                        ipb[:, h:h + 1], None, op0=A.add)

            # FFN weights stream while the AllGathers run
            w1T16 = ak.tile([P, 2 * DFF], DT16)
            nc.sync.dma_start(w1T16[:], w1T_d[:])
            w2T16 = ak.tile([P, (DFF // P) * D], DT16)
            nc.sync.dma_start(w2T16[:], w2T_d[:])

            # residual + out_proj bias, pre-added (Vector idle here)
            hbo = ak.tile([P, MPC * D], DT32)
            nc.vector.tensor_tensor(
                hbo[:].rearrange("p (m d) -> p m d", m=MPC),
                h_t[:].rearrange("p (m d) -> p m d", m=MPC),
                bc4(bo_bc), op=A.add)

            # ---- load gathered K^T / V (per-core simple blocks so the
            # first S / PV matmuls start on block 0 while later blocks
            # are still in flight) ----
            kT16 = ak.tile([P, H * N], DT8)
            v16 = ak.tile([P, H * N], DT8)
            gkv = kv_gath[:].rearrange("(g x p) n -> g x p n",
                                       g=N_CORES, x=4, p=P)
            for g in range(N_CORES):
                for h in range(H):
                    nc.sync.dma_start(
                        kT16[:, N * h + NPC * g:N * h + NPC * (g + 1)],
                        gkv[g, h])
                for h in range(H):
                    nc.sync.dma_start(
                        v16[:, N * h + NPC * g:N * h + NPC * (g + 1)],
                        gkv[g, 2 + h])

            # ---- interleaved S^T -> exp -> PV + sums (V arrives with K) ----
            oT16 = ak.tile([P, H * NPC], DT16)
            recT = ak.tile([P, H * MPC], DT32)
            esum = [ak.tile([P, 2 * NPC], DT16, tag=f"esum{h}",
                            name=f"esum{h}") for h in range(H)]
            KT2 = KT // 2
            with tc.tile_pool(name="att_es", bufs=4) as esb, \
                 tc.tile_pool(name="att_sb", bufs=2) as atsb, \
                 tc.tile_pool(name="att_ps", bufs=1, space="PSUM") as atps, \
                 tc.tile_pool(name="s_ps", bufs=2, space="PSUM") as sps:
                o_ps = [atps.tile([P, NPC], DT32, space="PSUM",
                                  tag=f"o{h}", name=f"o{h}")
                        for h in range(H)]

                # PV lags one sub-iteration behind S/exp so exp(i) runs on
                # the Act engine while the in-order PE does PV(i-1)
                pv_prev = None

                def emit_pv(kt2, h, es):
                    for u in range(2):
                        kt = 2 * kt2 + u
                        nc.tensor.matmul(
                            o_ps[h][:],
                            lhsT=v16[:, N * h + P * kt:N * h + P * (kt + 1)],
                            rhs=es[:, NPC * u:NPC * (u + 1)],
                            start=(kt == 0), stop=(kt == KT - 1))

                for kt2 in range(KT2):
                    for h in range(H):
                        ps_s = sps.tile([P, 2 * NPC], DT32, space="PSUM",
                                        tag="S")
                        for u in range(2):
                            kt = 2 * kt2 + u
                            nc.tensor.matmul(
                                ps_s[:, NPC * u:NPC * (u + 1)],
                                lhsT=kT16[:, N * h + P * kt:N * h + P * (kt + 1)],
                                rhs=qT16[:, NPC * h:NPC * (h + 1)],
                                start=True, stop=True)
                        es = esb.tile([P, 2 * NPC], DT8, tag="es")
                        nc.scalar.activation(es[:], ps_s[:], F.Exp,
                                             scale=INV_SQRT_DH)
                        if pv_prev is not None:
                            emit_pv(*pv_prev)
                        pv_prev = (kt2, h, es)
                        if kt2 == 0:
                            nc.vector.tensor_copy(esum[h][:], es[:])
                        else:
                            nc.vector.tensor_add(esum[h][:], esum[h][:], es[:])
                emit_pv(*pv_prev)
                # prefetch the sqrt table (for LN1/LN2) under the PV tail
                nc.scalar.activation(dummy_e[:], dinv[:, 0:1], F.Sqrt)

                # denominators: fold esum halves, one matmul per head, and
                # transpose to per-partition reciprocals — emitted before the
                # tail PVs so the Vector/transpose chain hides under them
                sum_ps = [atps.tile([1, NPC], DT32, space="PSUM",
                                    tag=f"sm{h}", name=f"sm{h}")
                          for h in range(H)]
                for h in range(H):
                    nc.vector.tensor_add(esum[h][:, 0:NPC], esum[h][:, 0:NPC],
                                         esum[h][:, NPC:2 * NPC])
                    nc.tensor.matmul(sum_ps[h][:], lhsT=ones16_col[:],
                                     rhs=esum[h][:, 0:NPC],
                                     start=True, stop=True)
                    srow = atsb.tile([1, NPC], DT32, tag="srow")
                    nc.vector.tensor_copy(srow[:], sum_ps[h][:])
                    sT_ps = sps.tile([P, MPC], DT32, space="PSUM", tag="S",
                                     name="sTps")
                    for m in range(MPC):
                        nc.tensor.transpose(
                            sT_ps[:, m:m + 1], srow[:, P * m:P * (m + 1)],
                            ident[0:1, 0:1])
                    nc.vector.reciprocal(recT[:, MPC * h:MPC * (h + 1)],
                                         sT_ps[:])

                # copy unnormalized o to sbuf
                for h in range(H):
                    nc.vector.tensor_copy(oT16[:, NPC * h:NPC * (h + 1)],
                                          o_ps[h][:])

            # ---- o_proj + residual + LN1, pipelined per m-chunk ----
            h1_t = ak.tile([P, MPC * D], DT32)
            h1T16 = ak.tile([P, 2 * NPC], DT16)
            with tc.tile_pool(name="ln_sb", bufs=2) as lsb:

                def layernorm_m(dst, x_m, g_sl, b_sl, tag):
                    """LN over feature dim for one [128, D] chunk.
                    mean/var via DVE bn_stats so the Act engine only ever
                    needs Sqrt (no Square -> no act-table thrash)."""
                    stats = lsb.tile([P, 6], DT32, tag=f"{tag}st")
                    nc.vector.bn_stats(out=stats[:], in_=x_m)
                    mv = lsb.tile([P, 2], DT32, tag=f"{tag}mv")
                    nc.vector.bn_aggr(out=mv[:], in_=stats[:])
                    negmu = lsb.tile([P, 1], DT32, tag=f"{tag}nm")
                    nc.vector.tensor_scalar(negmu[:], mv[:, 0:1], -1.0, None,
                                            op0=A.mult)
                    var = lsb.tile([P, 1], DT32, tag=f"{tag}vr")
                    nc.vector.tensor_scalar(var[:], mv[:, 1:2], EPS, None,
                                            op0=A.add)
                    sd = lsb.tile([P, 1], DT32, tag=f"{tag}sd")
                    nc.scalar.activation(sd[:], var[:], F.Sqrt)
                    rstd = lsb.tile([P, 1], DT32, tag=f"{tag}rs")
                    nc.vector.reciprocal(rstd[:], sd[:])
                    xc = lsb.tile([P, D], DT32, tag=f"{tag}xc")
                    nc.vector.tensor_scalar(xc[:], x_m, negmu[:], rstd[:],
                                            op0=A.add, op1=A.mult)
                    nc.vector.tensor_tensor(xc[:], xc[:], g_sl, op=A.mult)
                    nc.vector.tensor_tensor(dst, xc[:], b_sl, op=A.add)

                # all o_proj matmuls first (PE stays busy), then per-m
                # normalize + residual + LN1 + transpose, pipelined
                ctx_op = tc.tile_pool(name="op_ps", bufs=1, space="PSUM")
                ops = ctx_op.__enter__()
                ctx_tp2 = tc.tile_pool(name="tr2_ps", bufs=2, space="PSUM")
                tps2 = ctx_tp2.__enter__()
                pa = []
                for m in range(MPC):
                    pm = ops.tile([P, 2 * D], DT32, space="PSUM", tag="op",
                                  bufs=4, name=f"pa{m}")
                    for h in range(H):
                        nc.tensor.matmul(
                            pm[:, D * h:D * (h + 1)],
                            lhsT=oT16[:, NPC * h + P * m:NPC * h + P * (m + 1)],
                            rhs=woT16[:, D * h:D * (h + 1)],
                            start=True, stop=True)
                    pa.append(pm)
                for m in range(MPC):
                    x1m = lsb.tile([P, D], DT32, tag="x1m")
                    nc.vector.tensor_scalar(x1m[:], pa[m][:, 0:D],
                                            recT[:, m:m + 1], None,
                                            op0=A.mult)
                    nc.vector.scalar_tensor_tensor(
                        x1m[:], pa[m][:, D:2 * D],
                        recT[:, MPC + m:MPC + m + 1],
                        x1m[:], op0=A.mult, op1=A.add)
                    nc.vector.tensor_add(x1m[:], x1m[:],
                                         hbo[:, D * m:D * (m + 1)])
                    layernorm_m(h1_t[:, D * m:D * (m + 1)], x1m[:],
                                ln1g_bc, ln1b_bc, "a")
                    for f in range(2):
                        ptr = tps2.tile([P, P], DT32, space="PSUM", tag="tr2")
                        nc.tensor.transpose(
                            ptr[:],
                            h1_t[:, D * m + P * f:D * m + P * (f + 1)],
                            ident[:])
                        nc.vector.tensor_copy(
                            h1T16[:, NPC * f + P * m:NPC * f + P * (m + 1)],
                            ptr[:])
                ctx_tp2.__exit__(None, None, None)
                ctx_op.__exit__(None, None, None)

                # ---- FFN ----
                ff1T = ak.tile([P, (DFF // P) * NPC], DT16)
                with tc.tile_pool(name="f1_ps", bufs=3, space="PSUM") as fps:
                    for dc in range(DFF // P):
                        pf = fps.tile([P, NPC], DT32, space="PSUM", tag="f1")
                        for k in range(2):
                            nc.tensor.matmul(
                                pf[:],
                                lhsT=w1T16[:, DFF * k + P * dc:
                                           DFF * k + P * (dc + 1)],
                                rhs=h1T16[:, NPC * k:NPC * (k + 1)],
                                start=(k == 0), stop=(k == 1))
                        nc.scalar.activation(
                            ff1T[:, NPC * dc:NPC * (dc + 1)], pf[:], F.Relu,
                            bias=b1t[:, dc:dc + 1])

                with tc.tile_pool(name="f2_ps", bufs=2, space="PSUM") as fps2:
                    for m in range(MPC):
                        pf2 = fps2.tile([P, D], DT32, space="PSUM", tag="f2")
                        for kt2 in range(DFF // P):
                            nc.tensor.matmul(
                                pf2[:],
                                lhsT=ff1T[:, NPC * kt2 + P * m:
                                          NPC * kt2 + P * (m + 1)],
                                rhs=w2T16[:, D * kt2:D * (kt2 + 1)],
                                start=(kt2 == 0), stop=(kt2 == DFF // P - 1))
                        x2m = lsb.tile([P, D], DT32, tag="x2m")
                        nc.vector.scalar_tensor_tensor(
                            x2m[:], pf2[:], 1.0,
                            h1_t[:, D * m:D * (m + 1)], op0=A.mult, op1=A.add)
                        nc.vector.tensor_tensor(x2m[:], x2m[:], b2_bc,
                                                op=A.add)
                        out_m = lsb.tile([P, D], DT32, tag="outm")
                        layernorm_m(out_m[:], x2m[:], ln2g_bc, ln2b_bc, "b")
                        nc.scalar.dma_start(
                            out_d[:].rearrange("(m p) d -> m p d", p=P)[m],
                            out_m[:])
            ctx_attn.__exit__(None, None, None)

    nc.compile()
    return nc


# ======================= host-side prep =======================

def _prep_inputs(x, edge_index, edge_weight, W_gcn, b_gcn, in_proj_w,
                 in_proj_b, out_proj_w, out_proj_b, lin1_w, lin1_b, lin2_w,
                 lin2_b, ln1_g, ln1_b, ln2_g, ln2_b):
    """Pure index-permutation / layout prep. Returns per-core input maps."""
    x = np.asarray(x, np.float32)
    src = np.asarray(edge_index[0], np.int64)
    dst = np.asarray(edge_index[1], np.int64)
    w = np.asarray(edge_weight, np.float32)

    def wrap128(a):
        # [n*128, m] -> [128, n*m] with col block t <- rows [128t, 128t+128)
        n = a.shape[0] // P
        return np.ascontiguousarray(
            a.reshape(n, P, a.shape[1]).transpose(1, 0, 2).reshape(P, -1))

    ident = np.eye(P, dtype=np.float32)
    # out_proj bias with the V-bias term folded in (softmax rows sum to 1)
    bv = np.asarray(in_proj_b, np.float32)[2 * D:3 * D]
    bo_eff = (np.asarray(out_proj_b, np.float32)
              + bv @ np.asarray(out_proj_w, np.float32).T)
    bias_row = np.concatenate([
        np.asarray(v, np.float32).reshape(-1) for v in
        (b_gcn, lin2_b, ln1_g, ln1_b, ln2_g, ln2_b, bo_eff)
    ]).reshape(1, -1)
    bias_stack = np.ascontiguousarray(np.tile(bias_row, (P, 1)))

    f16 = np.float16
    shared = {
        "wg": wrap128(np.asarray(W_gcn, np.float32)).astype(f16),
        "ident": ident,
        "winT": wrap128(np.ascontiguousarray(
            np.asarray(in_proj_w, np.float32).T)).astype(f16),
        "ipb": np.ascontiguousarray(
            np.asarray(in_proj_b, np.float32).reshape(6, P).T),
        "woT": wrap128(np.ascontiguousarray(
            np.asarray(out_proj_w, np.float32).T)).astype(f16),
        "w1T": wrap128(np.ascontiguousarray(
            np.asarray(lin1_w, np.float32).T)).astype(f16),
        "b1": np.ascontiguousarray(
            np.asarray(lin1_b, np.float32).reshape(DFF // P, P).T),
        "w2T": wrap128(np.ascontiguousarray(
            np.asarray(lin2_w, np.float32).T)).astype(f16),
        "bias": bias_stack,
    }

    # j-major wrap of x.T: column block 512*j + 128*k holds the lhsT tile
    # for node block j, din block k -> xw[j] only needs its own 1 KB/row
    # prefix of the stream, so matmuls start while the DMA is in flight
    shared_xTf = np.ascontiguousarray(
        x.reshape(KT, P, DIN // P, P)          # [j, c, k, p]
         .transpose(3, 0, 2, 1)                # [p, j, k, c]
         .reshape(P, -1)).astype(f16)
    # full per-dst weight table for replicated degree computation
    wbdf = np.zeros((N, KBD), np.float32)
    cntf = np.zeros(N, np.int32)
    for di, wi in zip(dst.tolist(), w.tolist()):
        j = int(cntf[di])
        assert j < KBD
        wbdf[di, j] = wi
        cntf[di] = j + 1
    wbdf_full_w = wrap128(wbdf).astype(f16)

    core_of = dst // NPC
    in_maps = []
    for c in range(N_CORES):
        sel = np.nonzero(core_of == c)[0]
        s_c = src[sel]
        d_c = (dst[sel] - NPC * c).astype(np.int64)
        w_c = w[sel]

        w_arr = np.zeros((N, KPAD), np.float32)
        idx_arr = np.full((N, KPAD), -1, np.int16)
        counts = np.zeros(N, np.int32)
        first_slot = {}
        for si, di, wi in zip(s_c.tolist(), d_c.tolist(), w_c.tolist()):
            key = si * NPC + di
            slot = first_slot.get(key)
            if slot is None:
                j = int(counts[si])
                assert j < KPAD, f"KPAD overflow at src {si}"
                counts[si] = j + 1
                w_arr[si, j] = wi
                # scatter calls fuse two k-tiles: odd tiles write the upper
                # 512-element half of the fused output
                idx_arr[si, j] = di + NPC * ((si // P) % 2)
                first_slot[key] = j
            else:
                # duplicate (src, dst) edge: fold its weight into the first
                # slot so the on-device scatter sees unique indices
                w_arr[si, slot] += wi

        wbd = np.zeros((NPC, KBD), np.float32)
        cnt2 = np.zeros(NPC, np.int32)
        for di, wi in zip(d_c.tolist(), w_c.tolist()):
            j = int(cnt2[di])
            assert j < KBD, f"KBD overflow at dst {di}"
            wbd[di, j] = wi
            cnt2[di] = j + 1

        in_maps.append({
            **shared,
            "xT": wrap128(np.ascontiguousarray(
                x[NPC * c:NPC * (c + 1)].T)).astype(f16),
            "xTf": shared_xTf,
            "wbdf": wbdf_full_w,
            "warr": wrap128(w_arr).astype(f16),
            "idx": wrap128(idx_arr),
            "wbd": wrap128(wbd).astype(f16),
        })
    return in_maps


# ======================= runner =======================

class _Runner:
    """Persistent-jit SPMD executor (mirrors bass2jax.run_bass_via_pjrt)."""

    def __init__(self, nc):
        import jax
        from jax.sharding import Mesh, PartitionSpec
        from jax.experimental.shard_map import shard_map
        from concourse.bass2jax import (_bass_exec_p, install_neuronx_cc_hook,
                                        partition_id_tensor)
        install_neuronx_cc_hook()
        self.jax = jax
        partition_name = (nc.partition_id_tensor.name
                          if nc.partition_id_tensor else None)
        in_names, out_names, out_avals, zero_outs = [], [], [], []
        for alloc in nc.m.functions[0].allocations:
            if not isinstance(alloc, mybir.MemoryLocationSet):
                continue
            name = alloc.memorylocations[0].name
            if alloc.kind == "ExternalInput":
                if name != partition_name:
                    in_names.append(name)
            elif alloc.kind == "ExternalOutput":
                out_names.append(name)
                shape = tuple(alloc.tensor_shape)
                dtype = mybir.dt.np(alloc.dtype)
                out_avals.append(jax.core.ShapedArray(shape, dtype))
                zero_outs.append(np.zeros(shape, dtype))
        self.in_names, self.out_names = in_names, out_names
        self.out_shapes = [tuple(a.shape) for a in out_avals]
        self.n_params = len(in_names)
        self.zero_outs = zero_outs
        all_in = in_names + out_names
        if partition_name is not None:
            all_in.append(partition_name)

        def _body(*args):
            operands = list(args)
            if partition_name is not None:
                operands.append(partition_id_tensor())
            return tuple(_bass_exec_p.bind(
                *operands, out_avals=tuple(out_avals), in_names=tuple(all_in),
                out_names=tuple(out_names), lowering_input_output_aliases=(),
                sim_require_finite=True, sim_require_nnan=True, nc=nc))

        devices = jax.devices()[:N_CORES]
        self.mesh = Mesh(np.asarray(devices), ("core",))
        nin = self.n_params + len(out_names)
        self.fn = jax.jit(
            shard_map(_body, mesh=self.mesh,
                      in_specs=(PartitionSpec("core"),) * nin,
                      out_specs=(PartitionSpec("core"),) * len(out_names),
                      check_rep=False),
            keep_unused=True)

    def place(self, in_maps):
        import jax
        from jax.sharding import PartitionSpec
        per_core = [[np.asarray(m[n]) for n in self.in_names] for m in in_maps]
        concat = [np.concatenate([per_core[c][i] for c in range(N_CORES)], axis=0)
                  for i in range(self.n_params)]
        zeros = [np.zeros((N_CORES * z.shape[0], *z.shape[1:]), z.dtype)
                 for z in self.zero_outs]
        sh = jax.sharding.NamedSharding(self.mesh, PartitionSpec("core"))
        return [jax.device_put(a, sh) for a in (*concat, *zeros)]

    def run(self, args):
        outs = self.fn(*args)
        self.jax.block_until_ready(outs)
        return outs

    def results(self, outs):
        res = []
        for c in range(N_CORES):
            d = {}
            for i, name in enumerate(self.out_names):
                full = np.asarray(outs[i])
                ps = self.out_shapes[i]
                d[name] = full.reshape((N_CORES,) + ps)[c]
            res.append(d)
        return res


_CACHE = {}


def _get_runner():
    if "runner" not in _CACHE:
        nc = build_kernel()
        _CACHE["nc"] = nc
        _CACHE["runner"] = _Runner(nc)
    return _CACHE["runner"]


def kernel(**inputs) -> np.ndarray:
    runner = _get_runner()
    in_maps = _prep_inputs(**inputs)
    args = runner.place(in_maps)
    outs = runner.run(args)
    res = runner.results(outs)
    return np.concatenate([res[c]["out"] for c in range(N_CORES)], axis=0)
